# revision 14
# baseline (speedup 1.0000x reference)
"""GRU-with-resets Trainium2 kernel.

Strategy: resets chop each batch column's 512-step recurrence into
independent segments (mean length ~2, max ~20). Segments are strip-packed
into 128-wide "slot columns"; wave k processes step k of every packed
slot as one fused K=2048 GEMM per 128-slot tile:

    [i_r|i_z] += insT_tile.T @ W_i[:, rz]   (K-tiles 0..7)
    [h_r|h_z] += H_T_tile.T  @ W_h[:, rz]   (K-tiles 8..15, same PSUM)
    i_n, h_n accumulated in separate PSUM tiles (r gates h_n before tanh)

Gates run on ACT/DVE in row-major layout; the next-wave transposed state
H_T is produced by PE transposes of the masked fp16 new state. A per-row
mask (0 when a segment ends) zeroes dead slots so reused slots start from
h=0, which implements the reset semantics with no control flow.

All matmul operands are fp16 (PSUM accumulates fp32). 8 cores are pure
data-parallel over slots - no collectives.
"""

import math
import time

import numpy as np

T, B, H = 512, 64, 1024
NCORES = 8
P = 128  # partitions / tile width

_rt_info = {}  # filled by kernel(): timings etc. (for test harness use)


# ----------------------------------------------------------------------------
# Host-side scheduling
# ----------------------------------------------------------------------------

def _extract_segments(resets):
    """resets [T, B] bool -> list of (length, b, t0), the independent chains."""
    segs = []
    for b in range(resets.shape[1]):
        rs = np.flatnonzero(resets[1:, b]) + 1
        starts = np.concatenate([[0], rs]).astype(np.int64)
        ends = np.append(starts[1:], resets.shape[0])
        for t0, e in zip(starts, ends):
            segs.append((int(e - t0), int(b), int(t0)))
    return segs


def _pack_core(segs_desc, l_max):
    """Pack one core's segments into slot stacks.

    Cost model: every (wave, 128-slot tile) with any step present costs one
    unit (half if all rows are segment starts), so 1) commit as few
    tile-waves as possible, 2) fill the free slot-waves inside committed
    ones, 3) make leftover columns uniform-length so whole tiles are
    starts-only ("light": h-side matmul skipped; len-1 stacks never break
    lightness).

    Returns flat list of slots [end_wave, items]; items: (b, t0, L, w0).
    Occupancy intervals [w0, w0+L) within a slot may have gaps (mask keeps
    the state zeroed through gaps).
    """
    buckets = {}
    for (L, b, t0) in segs_desc:
        buckets.setdefault(L, []).append((b, t0))

    def new_slot():
        return [0, []]  # [end_wave, items]

    def push(slot, L, item, w0=None):
        b, t0 = item
        if w0 is None:
            w0 = slot[0]
        slot[1].append((b, t0, L, w0))
        slot[0] = max(slot[0], w0 + L)

    # --- 1. ragged FFD for long segments ---
    ragged = []
    for L in sorted([x for x in buckets if x >= 5], reverse=True):
        for item in buckets.pop(L):
            for s in ragged:
                if s[0] + L <= l_max:
                    push(s, L, item)
                    break
            else:
                s = new_slot()
                push(s, L, item)
                ragged.append(s)

    columns = []
    if ragged:
        for i in range(0, len(ragged), P):
            col = ragged[i:i + P]
            col += [new_slot() for _ in range(P - len(col))]
            columns.append(col)

    # --- 2. fill free windows of committed columns (height = col max) ---
    rem_lens = sorted([x for x in buckets if buckets[x]], reverse=True)
    for col in columns:
        h = max(s[0] for s in col)
        for s in col:
            while True:
                gap = h - s[0]
                L = next((x for x in rem_lens if x <= gap and buckets.get(x)),
                         0)
                if L == 0:
                    break
                push(s, L, buckets[L].pop())
        rem_lens = [x for x in rem_lens if buckets.get(x)]

    # --- 3. uniform columns for remaining lengths (len-1 fills) ---
    for L in sorted([x for x in buckets if x > 1 and buckets[x]],
                    reverse=True):
        items = buckets.pop(L)
        while items:
            k = min(max(1, l_max // L), math.ceil(len(items) / P))
            height = L * k
            col = []
            while items and len(col) < P:
                s = new_slot()
                for _ in range(min(k, len(items))):
                    push(s, L, items.pop())
                col.append(s)
            ones = buckets.get(1, [])
            while ones and len(col) < P:
                s = new_slot()
                for _ in range(min(height, len(ones))):
                    push(s, 1, ones.pop())
                col.append(s)
            col += [new_slot() for _ in range(P - len(col))]
            columns.append(col)

    # --- 4. leftover len-1 columns (all tiles light) ---
    ones = buckets.pop(1, [])
    while ones:
        k = math.ceil(len(ones) / P)
        col = []
        while ones and len(col) < P:
            s = new_slot()
            for _ in range(min(k, len(ones))):
                push(s, 1, ones.pop())
            col.append(s)
        col += [new_slot() for _ in range(P - len(col))]
        columns.append(col)

    # order columns by height desc, flatten to slot list
    columns.sort(key=lambda col: -max(s[0] for s in col))
    slots = []
    for col in columns:
        slots.extend(col)
    # downstream expects [cap, used, items]
    return [[l_max, s[0], s[1]] for s in slots]


def _schedule(resets):
    """Pack segments into per-core slot columns.

    Returns dict with common profile (tiles per wave), per-core row maps.
    """
    segs = _extract_segments(resets)
    segs.sort(key=lambda s: (-s[0], s[1], s[2]))
    l_max = segs[0][0]

    # balance across cores by total rows
    core_segs = [[] for _ in range(NCORES)]
    load = [0] * NCORES
    for s in segs:
        c = min(range(NCORES), key=lambda i: (load[i], i))
        core_segs[c].append(s)
        load[c] += s[0]

    # Per-core packing into 128-slot columns. Uniform same-length columns
    # keep whole tiles "all-starts" (light program: no h-side matmul);
    # lengths >= 5 go to ragged FFD columns topped up with short filler.
    packs = [_pack_core(core_segs[c], l_max) for c in range(NCORES)]

    # common per-wave tile counts: a slot is occupied over waves [0, used)
    nw = l_max
    nt = [0] * nw  # tiles per wave
    for c in range(NCORES):
        for i, s in enumerate(packs[c]):
            for k in range(s[1]):
                nt[k] = max(nt[k], i // P + 1)
    offs = np.concatenate([[0], np.cumsum([n * P for n in nt])]).astype(np.int64)
    r_total = int(offs[-1])

    # per-core row maps + light-tile flags
    cores = []
    light = [[True] * nt[k] for k in range(nw)]
    for c in range(NCORES):
        src = np.full(r_total, -1, dtype=np.int64)
        mask = np.zeros(r_total, dtype=np.float32)
        ydst = np.full(r_total, -1, dtype=np.int64)
        for i, s in enumerate(packs[c]):
            for (b, t0, L, w0) in s[2]:
                for j in range(L):
                    k = w0 + j
                    pos = int(offs[k]) + i
                    src[pos] = (t0 + j) * B + b
                    mask[pos] = 1.0 if j < L - 1 else 0.0
                    ydst[pos] = (t0 + j) * B + b
                    if j > 0:
                        light[k][i // P] = False
        cores.append({"src": src, "mask": mask, "ydst": ydst})

    # sanity: every (t,b) exactly once
    allsrc = np.concatenate([c["src"] for c in cores])
    allsrc = np.sort(allsrc[allsrc >= 0])
    assert allsrc.size == T * B and np.array_equal(allsrc, np.arange(T * B)), (
        "schedule does not cover all timesteps exactly once"
    )

    return {
        "nw": nw,
        "nt": nt,
        "offs": offs,
        "r_total": r_total,
        "cores": cores,
        "light": light,
        "n_units": int(sum(nt)),
    }


# ----------------------------------------------------------------------------
# Bass kernel builder
# ----------------------------------------------------------------------------

def _build_nc(sched, use_bi, use_bhn):
    import concourse.bass as bass
    import concourse.tile as tile
    from concourse import bacc, mybir

    f32 = mybir.dt.float32
    f16 = mybir.dt.float16
    AF = mybir.ActivationFunctionType

    nw, nt, offs, r_total = sched["nw"], sched["nt"], sched["offs"], sched["r_total"]
    light = sched["light"]
    ncols = nt[0]  # slot columns
    S = ncols * P  # total slots

    nc = bacc.Bacc("TRN2", target_bir_lowering=False, debug=False,
                   num_devices=NCORES)

    insT_d = nc.dram_tensor("insT", [H, r_total], f32, kind="ExternalInput").ap()
    wi_d = nc.dram_tensor("Wi", [H, 3 * H], f32, kind="ExternalInput").ap()
    wh_d = nc.dram_tensor("Wh", [H, 3 * H], f32, kind="ExternalInput").ap()
    mask_d = nc.dram_tensor("mask", [r_total, 1], f32, kind="ExternalInput").ap()
    if use_bi:
        bib_d = nc.dram_tensor("bib", [P, 3 * H], f32, kind="ExternalInput").ap()
    if use_bhn:
        bhnb_d = nc.dram_tensor("bhnb", [P, H], f32, kind="ExternalInput").ap()
    y_d = nc.dram_tensor("Y", [r_total, H], f32, kind="ExternalOutput").ap()

    with tile.TileContext(nc) as tc:
        with (
            tc.tile_pool(name="const", bufs=1) as constp,
            tc.tile_pool(name="wtmp", bufs=2) as wtmpp,
            tc.tile_pool(name="state", bufs=1) as statep,
            tc.tile_pool(name="xin", bufs=3) as xinp,
            tc.tile_pool(name="gates", bufs=2) as gatesp,
            tc.tile_pool(name="hout", bufs=2) as houtp,
            tc.tile_pool(name="psum", bufs=2, space="PSUM") as psum,
        ):
            # --- constants: weights (fp16), identity, biases ---
            w16 = constp.tile([P, 16 * 3 * H], f16, tag="w16")
            for kk in range(16):
                src = wi_d if kk < 8 else wh_d
                r0 = (kk % 8) * P
                wt = wtmpp.tile([P, 3 * H], f32, tag="wtmp")
                nc.sync.dma_start(wt[:], src[r0:r0 + P, :])
                nc.vector.tensor_copy(w16[:, kk * 3 * H:(kk + 1) * 3 * H], wt[:])

            if use_bi:
                bib = constp.tile([P, 3 * H], f32, tag="bib")
                nc.sync.dma_start(bib[:], bib_d[:, :])
            if use_bhn:
                bhnb = constp.tile([P, H], f32, tag="bhnb")
                nc.sync.dma_start(bhnb[:], bhnb_d[:, :])

            # --- persistent state ---
            # transposed state: per K-chunk kk, columns = slots
            ht = statep.tile([P, 8 * S], f16, tag="ht")
            nc.vector.memset(ht[:], 0.0)
            # row-major state per slot-column tile
            hr = statep.tile([P, ncols * H], f16, tag="hr")
            nc.vector.memset(hr[:], 0.0)

            def wcol(kk, gate, jc):
                # rhs slice for W k-tile kk, gate in {0:r, 1:z, 2:n}, half jc
                c0 = kk * 3 * H + gate * H + jc * 512
                return w16[:, c0:c0 + 512]

            insT_r = insT_d.rearrange("(a p) r -> p a r", p=P)

            for k in range(nw):
                for t in range(nt[k]):
                    rows0 = int(offs[k]) + t * P
                    slot0 = t * P
                    is_light = light[k][t]

                    # load + cast ins lhsT K-tiles: [128, 8, 128]
                    xf = xinp.tile([P, 8, P], f32, tag="xf")
                    nc.sync.dma_start(xf[:], insT_r[:, :, rows0:rows0 + P])
                    xh = xinp.tile([P, 8, P], f16, tag="xh")
                    nc.vector.tensor_copy(xh[:], xf[:])

                    mk = xinp.tile([P, 1], f32, tag="mk")
                    nc.sync.dma_start(mk[:], mask_d[rows0:rows0 + P, :])

                    def lhsT(kk):
                        if kk < 8:
                            return xh[:, kk, :]
                        c = (kk - 8) * S + slot0
                        return ht[:, c:c + P]

                    hfull = houtp.tile([P, H], f32, tag="hfull")

                    for jc in range(2):
                        nkt = 8 if is_light else 16
                        pr = psum.tile([P, 512], f32, tag="pr")
                        for kk in range(nkt):
                            nc.tensor.matmul(pr[:], lhsT(kk), wcol(kk, 0, jc),
                                             start=(kk == 0), stop=(kk == nkt - 1))
                        pz = psum.tile([P, 512], f32, tag="pz")
                        for kk in range(nkt):
                            nc.tensor.matmul(pz[:], lhsT(kk), wcol(kk, 1, jc),
                                             start=(kk == 0), stop=(kk == nkt - 1))
                        pin = psum.tile([P, 512], f32, tag="pin")
                        for kk in range(8):
                            nc.tensor.matmul(pin[:], xh[:, kk, :], wcol(kk, 2, jc),
                                             start=(kk == 0), stop=(kk == 7))
                        if not is_light:
                            phn = psum.tile([P, 512], f32, tag="phn")
                            for kk in range(8):
                                nc.tensor.matmul(phn[:], lhsT(kk + 8),
                                                 wcol(kk + 8, 2, jc),
                                                 start=(kk == 0), stop=(kk == 7))

                        if use_bi:
                            nc.vector.tensor_add(pr[:], pr[:],
                                                 bib[:, 0 * H + jc * 512:0 * H + jc * 512 + 512])
                            nc.vector.tensor_add(pz[:], pz[:],
                                                 bib[:, 1 * H + jc * 512:1 * H + jc * 512 + 512])
                            nc.vector.tensor_add(pin[:], pin[:],
                                                 bib[:, 2 * H + jc * 512:2 * H + jc * 512 + 512])

                        r_t = gatesp.tile([P, 512], f32, tag="r")
                        nc.scalar.activation(r_t[:], pr[:], AF.Sigmoid)
                        z_t = gatesp.tile([P, 512], f32, tag="z")
                        nc.scalar.activation(z_t[:], pz[:], AF.Sigmoid)

                        n_t = gatesp.tile([P, 512], f32, tag="n")
                        if is_light:
                            # h == 0: n = tanh(i_n + r * b_hn)
                            if use_bhn:
                                t1 = gatesp.tile([P, 512], f32, tag="t1")
                                nc.vector.tensor_mul(
                                    t1[:], r_t[:], bhnb[:, jc * 512:jc * 512 + 512])
                                t2 = gatesp.tile([P, 512], f32, tag="t2")
                                nc.vector.tensor_add(t2[:], t1[:], pin[:])
                                nc.scalar.activation(n_t[:], t2[:], AF.Tanh)
                            else:
                                nc.scalar.activation(n_t[:], pin[:], AF.Tanh)
                            # h = (1-z)*n = n - z*n
                            zs = gatesp.tile([P, 512], f32, tag="zs")
                            nc.vector.tensor_mul(zs[:], z_t[:], n_t[:])
                            nc.vector.tensor_sub(
                                hfull[:, jc * 512:jc * 512 + 512], n_t[:], zs[:])
                        else:
                            if use_bhn:
                                nc.vector.tensor_add(
                                    phn[:], phn[:], bhnb[:, jc * 512:jc * 512 + 512])
                            t1 = gatesp.tile([P, 512], f32, tag="t1")
                            nc.vector.tensor_mul(t1[:], r_t[:], phn[:])
                            t2 = gatesp.tile([P, 512], f32, tag="t2")
                            nc.vector.tensor_add(t2[:], t1[:], pin[:])
                            nc.scalar.activation(n_t[:], t2[:], AF.Tanh)
                            # h = n + z*(h_prev - n)
                            hp = hr[:, t * H + jc * 512:t * H + jc * 512 + 512]
                            s_t = gatesp.tile([P, 512], f32, tag="s")
                            nc.vector.tensor_sub(s_t[:], hp, n_t[:])
                            zs = gatesp.tile([P, 512], f32, tag="zs")
                            nc.vector.tensor_mul(zs[:], z_t[:], s_t[:])
                            nc.vector.tensor_add(
                                hfull[:, jc * 512:jc * 512 + 512], n_t[:], zs[:])

                        # masked fp16 state update (row-major)
                        nc.vector.tensor_scalar_mul(
                            hr[:, t * H + jc * 512:t * H + jc * 512 + 512],
                            hfull[:, jc * 512:jc * 512 + 512], mk[:])

                    # output
                    nc.sync.dma_start(y_d[rows0:rows0 + P, :], hfull[:])

                    # transposed state update for next wave (DMA xbar
                    # transpose into a contiguous staging tile; gpsimd copies
                    # it into the persistent H_T buffer)
                    if k + 1 < nw and t < nt[k + 1]:
                        for kk in range(8):
                            tp = xinp.tile([P, P], f16, tag="tp")
                            nc.sync.dma_start_transpose(
                                tp[:], hr[:, t * H + kk * P:t * H + (kk + 1) * P])
                            nc.gpsimd.tensor_copy(
                                ht[:, kk * S + slot0:kk * S + slot0 + P], tp[:])

    nc.compile()
    return nc


# ----------------------------------------------------------------------------
# Entry point
# ----------------------------------------------------------------------------

def kernel(ins, resets, W_i, b_i, W_h, b_hn):
    from concourse.bass_utils import run_bass_kernel_spmd

    t_host0 = time.time()
    ins = np.asarray(ins, dtype=np.float32)
    resets = np.asarray(resets).astype(bool)
    W_i = np.ascontiguousarray(np.asarray(W_i, dtype=np.float32))
    W_h = np.ascontiguousarray(np.asarray(W_h, dtype=np.float32))
    b_i = np.asarray(b_i, dtype=np.float32)
    b_hn = np.asarray(b_hn, dtype=np.float32)

    use_bi = bool(np.any(b_i != 0))
    use_bhn = bool(np.any(b_hn != 0))

    sched = _schedule(resets)
    r_total = sched["r_total"]

    t_build0 = time.time()
    nc = _build_nc(sched, use_bi, use_bhn)
    t_build = time.time() - t_build0

    ins_flat = ins.reshape(T * B, H)
    in_maps = []
    for c in range(NCORES):
        src = sched["cores"][c]["src"]
        a = np.zeros((r_total, H), dtype=np.float32)
        valid = src >= 0
        a[valid] = ins_flat[src[valid]]
        im = {
            "insT": np.ascontiguousarray(a.T),
            "Wi": W_i,
            "Wh": W_h,
            "mask": sched["cores"][c]["mask"].reshape(r_total, 1),
        }
        if use_bi:
            im["bib"] = np.broadcast_to(b_i, (P, 3 * H)).copy()
        if use_bhn:
            im["bhnb"] = np.broadcast_to(b_hn, (P, H)).copy()
        in_maps.append(im)

    t_run0 = time.time()
    res = run_bass_kernel_spmd(nc, in_maps, list(range(NCORES)))
    t_run = time.time() - t_run0

    ys_flat = np.zeros((T * B, H), dtype=np.float32)
    for c in range(NCORES):
        ydst = sched["cores"][c]["ydst"]
        valid = ydst >= 0
        ys_flat[ydst[valid]] = res.results[c]["Y"][valid]

    _rt_info.update(
        sched_units=sched["n_units"], r_total=r_total, nw=sched["nw"],
        nt=sched["nt"], t_build=t_build, t_run=t_run,
        t_host=time.time() - t_host0, nc=nc, in_maps=in_maps,
        exec_time_ns=res.exec_time_ns,
    )
    return ys_flat.reshape(T, B, H)


# ----------------------------------------------------------------------------
# Benchmarking (wall-clock of the compiled executable; dispatch overhead is
# estimated with a stub NEFF and subtracted)
# ----------------------------------------------------------------------------

def _make_callable(nc, in_maps, donate=False):
    """jit'd shard_map callable over the prebuilt bass module (all 8 cores).

    Returns (fn, dev_args): call fn(*dev_args) -> tuple of outputs.
    """
    import jax
    import numpy as np
    from jax.experimental.shard_map import shard_map
    from jax.sharding import Mesh, PartitionSpec

    from concourse import mybir
    from concourse.bass2jax import (
        _bass_exec_p,
        install_neuronx_cc_hook,
        partition_id_tensor,
    )

    install_neuronx_cc_hook()
    n_cores = len(in_maps)
    partition_name = (nc.partition_id_tensor.name
                      if nc.partition_id_tensor else None)

    in_names, out_names, out_avals, zero_outs = [], [], [], []
    for alloc in nc.m.functions[0].allocations:
        if not isinstance(alloc, mybir.MemoryLocationSet):
            continue
        name = alloc.memorylocations[0].name
        if alloc.kind == "ExternalInput":
            if name != partition_name:
                in_names.append(name)
        elif alloc.kind == "ExternalOutput":
            out_names.append(name)
            shape = tuple(alloc.tensor_shape)
            dtype = mybir.dt.np(alloc.dtype)
            out_avals.append(jax.core.ShapedArray(shape, dtype))
            zero_outs.append(np.zeros(shape, dtype))
    n_params = len(in_names)
    all_names = in_names + out_names
    if partition_name is not None:
        all_names = all_names + [partition_name]

    def _body(*args):
        operands = list(args)
        if partition_name is not None:
            operands.append(partition_id_tensor())
        outs = _bass_exec_p.bind(
            *operands,
            out_avals=tuple(out_avals),
            in_names=tuple(all_names),
            out_names=tuple(out_names),
            lowering_input_output_aliases=(),
            sim_require_finite=True,
            sim_require_nnan=True,
            nc=nc,
        )
        return tuple(outs)

    devices = jax.devices()[:n_cores]
    mesh = Mesh(np.asarray(devices), ("core",))
    specs = (PartitionSpec("core"),) * (n_params + len(out_names))
    fn = jax.jit(
        shard_map(_body, mesh=mesh, in_specs=specs,
                  out_specs=(PartitionSpec("core"),) * len(out_names),
                  check_rep=False),
        donate_argnums=(tuple(range(n_params, n_params + len(out_names)))
                        if donate else ()),
        keep_unused=True,
    )
    concat = [
        np.concatenate([np.asarray(in_maps[c][n]) for c in range(n_cores)],
                       axis=0)
        for n in in_names
    ] + [np.zeros((n_cores * z.shape[0], *z.shape[1:]), z.dtype)
         for z in zero_outs]
    dev_args = [jax.device_put(a) for a in concat]
    return fn, dev_args


def bench(n=6):
    """Returns dict with wall-clock stats for the last-built kernel."""
    import jax

    assert "nc" in _rt_info, "run kernel() first"

    def time_calls(fn, args, n):
        ts = []
        for _ in range(n):
            t0 = time.perf_counter()
            jax.block_until_ready(fn(*args))
            ts.append(time.perf_counter() - t0)
        return ts

    fn, dev_args = _make_callable(_rt_info["nc"], _rt_info["in_maps"])
    t_real = time_calls(fn, dev_args, n)

    # dispatch-overhead stub: trivial NEFF with tiny I/O
    stub = _stub_nc()
    stub_maps = [{"sx": np.zeros((P, P), np.float32)} for _ in range(NCORES)]
    fn_s, dev_s = _make_callable(stub, stub_maps)
    t_stub = time_calls(fn_s, dev_s, n)

    t_real_med = sorted(t_real)[len(t_real) // 2]
    t_stub_med = sorted(t_stub)[len(t_stub) // 2]
    est = (t_real_med - t_stub_med) * 1e9
    out = {
        "t_real_ms": [round(t * 1e3, 3) for t in t_real],
        "t_stub_ms": [round(t * 1e3, 3) for t in t_stub],
        "hw_exec_est_ns": int(est),
    }
    _rt_info["bench"] = out
    return out


def _stub_nc():
    import concourse.tile as tile
    from concourse import bacc, mybir

    nc = bacc.Bacc("TRN2", target_bir_lowering=False, debug=False,
                   num_devices=NCORES)
    x_d = nc.dram_tensor("sx", [P, P], mybir.dt.float32,
                         kind="ExternalInput").ap()
    y_d = nc.dram_tensor("sy", [P, P], mybir.dt.float32,
                         kind="ExternalOutput").ap()
    with tile.TileContext(nc) as tc:
        with tc.tile_pool(name="p", bufs=1) as pool:
            t = pool.tile([P, P], mybir.dt.float32)
            nc.sync.dma_start(t[:], x_d[:, :])
            nc.sync.dma_start(y_d[:, :], t[:])
    nc.compile()
    return nc


# revision 27
# speedup vs baseline: 48.9509x; 48.9509x over previous
"""GRU-with-resets Trainium2 kernel.

Strategy: resets chop each batch column's 512-step recurrence into
independent segments (mean length ~2, max ~20). Segments are strip-packed
into 128-wide "slot columns"; wave k processes step k of every packed
slot as one fused K=2048 GEMM per 128-slot tile:

    [i_r|i_z] += insT_tile.T @ W_i[:, rz]   (K-tiles 0..7)
    [h_r|h_z] += H_T_tile.T  @ W_h[:, rz]   (K-tiles 8..15, same PSUM)
    i_n, h_n accumulated in separate PSUM tiles (r gates h_n before tanh)

Gates run on ACT/DVE in row-major layout; the next-wave transposed state
H_T is produced by PE transposes of the masked fp16 new state. A per-row
mask (0 when a segment ends) zeroes dead slots so reused slots start from
h=0, which implements the reset semantics with no control flow.

All matmul operands are fp16 (PSUM accumulates fp32). 8 cores are pure
data-parallel over slots - no collectives.
"""

import math
import time

import numpy as np

T, B, H = 512, 64, 1024
NCORES = 8
P = 128  # partitions / tile width

_rt_info = {}  # filled by kernel(): timings etc. (for test harness use)


# ----------------------------------------------------------------------------
# Host-side scheduling
# ----------------------------------------------------------------------------

def _extract_segments(resets):
    """resets [T, B] bool -> list of (length, b, t0), the independent chains."""
    segs = []
    for b in range(resets.shape[1]):
        rs = np.flatnonzero(resets[1:, b]) + 1
        starts = np.concatenate([[0], rs]).astype(np.int64)
        ends = np.append(starts[1:], resets.shape[0])
        for t0, e in zip(starts, ends):
            segs.append((int(e - t0), int(b), int(t0)))
    return segs


def _pack_core(segs_desc, l_max):
    """Pack one core's segments into slot stacks.

    Cost model: every (wave, 128-slot tile) with any step present costs one
    unit (half if all rows are segment starts), so 1) commit as few
    tile-waves as possible, 2) fill the free slot-waves inside committed
    ones, 3) make leftover columns uniform-length so whole tiles are
    starts-only ("light": h-side matmul skipped; len-1 stacks never break
    lightness).

    Returns flat list of slots [end_wave, items]; items: (b, t0, L, w0).
    Occupancy intervals [w0, w0+L) within a slot may have gaps (mask keeps
    the state zeroed through gaps).
    """
    buckets = {}
    for (L, b, t0) in segs_desc:
        buckets.setdefault(L, []).append((b, t0))

    def new_slot():
        return [0, []]  # [end_wave, items]

    def push(slot, L, item, w0=None):
        b, t0 = item
        if w0 is None:
            w0 = slot[0]
        slot[1].append((b, t0, L, w0))
        slot[0] = max(slot[0], w0 + L)

    # --- 1. ragged FFD for long segments ---
    ragged = []
    for L in sorted([x for x in buckets if x >= 5], reverse=True):
        for item in buckets.pop(L):
            for s in ragged:
                if s[0] + L <= l_max:
                    push(s, L, item)
                    break
            else:
                s = new_slot()
                push(s, L, item)
                ragged.append(s)

    columns = []
    if ragged:
        for i in range(0, len(ragged), P):
            col = ragged[i:i + P]
            col += [new_slot() for _ in range(P - len(col))]
            columns.append(col)

    # --- 2. fill free windows of committed columns (height = col max) ---
    rem_lens = sorted([x for x in buckets if buckets[x]], reverse=True)
    for col in columns:
        h = max(s[0] for s in col)
        for s in col:
            while True:
                gap = h - s[0]
                L = next((x for x in rem_lens if x <= gap and buckets.get(x)),
                         0)
                if L == 0:
                    break
                push(s, L, buckets[L].pop())
        rem_lens = [x for x in rem_lens if buckets.get(x)]

    # --- 3. uniform columns for remaining lengths (len-1 fills) ---
    for L in sorted([x for x in buckets if x > 1 and buckets[x]],
                    reverse=True):
        items = buckets.pop(L)
        while items:
            k = min(max(1, l_max // L), math.ceil(len(items) / P))
            height = L * k
            col = []
            while items and len(col) < P:
                s = new_slot()
                for _ in range(min(k, len(items))):
                    push(s, L, items.pop())
                col.append(s)
            ones = buckets.get(1, [])
            while ones and len(col) < P:
                s = new_slot()
                for _ in range(min(height, len(ones))):
                    push(s, 1, ones.pop())
                col.append(s)
            col += [new_slot() for _ in range(P - len(col))]
            columns.append(col)

    # --- 4. leftover len-1 columns (all tiles light) ---
    ones = buckets.pop(1, [])
    while ones:
        k = math.ceil(len(ones) / P)
        col = []
        while ones and len(col) < P:
            s = new_slot()
            for _ in range(min(k, len(ones))):
                push(s, 1, ones.pop())
            col.append(s)
        col += [new_slot() for _ in range(P - len(col))]
        columns.append(col)

    # order columns by height desc, flatten to slot list
    columns.sort(key=lambda col: -max(s[0] for s in col))
    slots = []
    for col in columns:
        slots.extend(col)
    # downstream expects [cap, used, items]
    return [[l_max, s[0], s[1]] for s in slots]


def _schedule(resets):
    """Pack segments into per-core slot columns.

    Returns dict with common profile (tiles per wave), per-core row maps.
    """
    segs = _extract_segments(resets)
    segs.sort(key=lambda s: (-s[0], s[1], s[2]))
    l_max = segs[0][0]

    # balance across cores by total rows
    core_segs = [[] for _ in range(NCORES)]
    load = [0] * NCORES
    for s in segs:
        c = min(range(NCORES), key=lambda i: (load[i], i))
        core_segs[c].append(s)
        load[c] += s[0]

    # Per-core packing into 128-slot columns. Uniform same-length columns
    # keep whole tiles "all-starts" (light program: no h-side matmul);
    # lengths >= 5 go to ragged FFD columns topped up with short filler.
    packs = [_pack_core(core_segs[c], l_max) for c in range(NCORES)]

    # common per-wave tile counts: a slot is occupied over waves [0, used)
    nw = l_max
    nt = [0] * nw  # tiles per wave
    for c in range(NCORES):
        for i, s in enumerate(packs[c]):
            for k in range(s[1]):
                nt[k] = max(nt[k], i // P + 1)
    offs = np.concatenate([[0], np.cumsum([n * P for n in nt])]).astype(np.int64)
    r_total = int(offs[-1])

    # per-core row maps + light-tile flags
    cores = []
    light = [[True] * nt[k] for k in range(nw)]
    for c in range(NCORES):
        src = np.full(r_total, -1, dtype=np.int64)
        mask = np.zeros(r_total, dtype=np.float32)
        ydst = np.full(r_total, -1, dtype=np.int64)
        for i, s in enumerate(packs[c]):
            for (b, t0, L, w0) in s[2]:
                for j in range(L):
                    k = w0 + j
                    pos = int(offs[k]) + i
                    src[pos] = (t0 + j) * B + b
                    mask[pos] = 1.0 if j < L - 1 else 0.0
                    ydst[pos] = (t0 + j) * B + b
                    if j > 0:
                        light[k][i // P] = False
        cores.append({"src": src, "mask": mask, "ydst": ydst})

    # sanity: every (t,b) exactly once
    allsrc = np.concatenate([c["src"] for c in cores])
    allsrc = np.sort(allsrc[allsrc >= 0])
    assert allsrc.size == T * B and np.array_equal(allsrc, np.arange(T * B)), (
        "schedule does not cover all timesteps exactly once"
    )

    return {
        "nw": nw,
        "nt": nt,
        "offs": offs,
        "r_total": r_total,
        "cores": cores,
        "light": light,
        "n_units": int(sum(nt)),
    }


# ----------------------------------------------------------------------------
# Bass kernel builder
# ----------------------------------------------------------------------------

def _build_nc(sched, use_bi, use_bhn, repeat=1):
    import os
    from contextlib import ExitStack
    import concourse.bass as bass
    import concourse.tile as tile
    from concourse import bacc, mybir

    abl = set(os.environ.get("KABLATE", "").split(","))
    bufs = dict(x.split("=") for x in os.environ.get("KBUFS", "").split(";")
                if "=" in x)
    bx = int(bufs.get("x", 3))
    bg = int(bufs.get("g", 2))
    bp = int(bufs.get("p", 2))
    bh = int(bufs.get("h", 2))

    f32 = mybir.dt.float32
    f16 = mybir.dt.float16
    AF = mybir.ActivationFunctionType

    nw, nt, offs, r_total = sched["nw"], sched["nt"], sched["offs"], sched["r_total"]
    light = sched["light"]
    ncols = nt[0]  # slot columns
    S = ncols * P  # total slots

    nc = bacc.Bacc("TRN2", target_bir_lowering=False, debug=False,
                   num_devices=NCORES)

    insT_d = nc.dram_tensor("insT", [H, r_total], f32, kind="ExternalInput").ap()
    wi_d = nc.dram_tensor("Wi", [H, 3 * H], f32, kind="ExternalInput").ap()
    wh_d = nc.dram_tensor("Wh", [H, 3 * H], f32, kind="ExternalInput").ap()
    mask_d = nc.dram_tensor("mask", [r_total, 1], f32, kind="ExternalInput").ap()
    ident_d = nc.dram_tensor("ident", [P, P], f32, kind="ExternalInput").ap()
    if use_bi:
        bib_d = nc.dram_tensor("bib", [P, 3 * H], f32, kind="ExternalInput").ap()
    if use_bhn:
        bhnb_d = nc.dram_tensor("bhnb", [P, H], f32, kind="ExternalInput").ap()
    y_d = nc.dram_tensor("Y", [r_total, H], f32, kind="ExternalOutput").ap()

    with tile.TileContext(nc) as tc:
        with (
            tc.tile_pool(name="const", bufs=1) as constp,
            tc.tile_pool(name="wtmp", bufs=12) as wtmpp,
            tc.tile_pool(name="state", bufs=1) as statep,
            tc.tile_pool(name="xin", bufs=bx) as xinp,
            tc.tile_pool(name="gates", bufs=bg) as gatesp,
            tc.tile_pool(name="hout", bufs=bh) as houtp,
            tc.tile_pool(name="psum", bufs=bp, space="PSUM") as psum,
        ):
            # --- constants: weights (fp16), identity, biases ---
            # Chunked load in first-use order (r half 0, z half 0, ...) with
            # casts spread over DVE/ACT/GpSimd so the first matmuls are not
            # gated on the whole 25MB conversion.
            w16 = constp.tile([P, 16 * 3 * H], f16, tag="w16")
            cast_engs = [nc.vector, nc.scalar, nc.gpsimd]
            ci = 0
            if "now" in abl:
                nc.vector.memset(w16[:], 0.0)
            for (gate, jc) in (() if "now" in abl else
                               ((0, 0), (1, 0), (2, 0), (0, 1), (1, 1), (2, 1))):
                for kk in range(16):
                    src = wi_d if kk < 8 else wh_d
                    r0 = (kk % 8) * P
                    c0 = gate * H + jc * 512
                    wt = wtmpp.tile([P, 512], f32, tag="wtmp")
                    nc.sync.dma_start(wt[:], src[r0:r0 + P, c0:c0 + 512])
                    dst = w16[:, kk * 3 * H + c0:kk * 3 * H + c0 + 512]
                    eng = cast_engs[ci % 3]
                    ci += 1
                    if eng is nc.scalar:
                        eng.copy(dst, wt[:])
                    else:
                        eng.tensor_copy(dst, wt[:])

            if use_bi:
                bib = constp.tile([P, 3 * H], f32, tag="bib")
                nc.sync.dma_start(bib[:], bib_d[:, :])
            if use_bhn:
                bhnb = constp.tile([P, H], f32, tag="bhnb")
                nc.sync.dma_start(bhnb[:], bhnb_d[:, :])

            # --- persistent state ---
            # transposed state: per K-chunk kk, columns = slots
            ident = constp.tile([P, P], f32, tag="ident")
            nc.sync.dma_start(ident[:], ident_d[:, :])
            ht = statep.tile([P, 8 * S], f16, tag="ht")
            # row-major state per slot-column tile
            hr = statep.tile([P, ncols * H], f32, tag="hr")

            rep_ctx = ExitStack()
            if repeat > 1:
                rep_ctx.enter_context(tc.For_i(0, repeat, 1))
            nc.vector.memset(ht[:], 0.0)
            nc.vector.memset(hr[:], 0.0)

            def wcol(kk, gate, jc):
                # rhs slice for W k-tile kk, gate in {0:r, 1:z, 2:n}, half jc
                c0 = kk * 3 * H + gate * H + jc * 512
                return w16[:, c0:c0 + 512]

            insT_r = insT_d.rearrange("(a p) r -> p a r", p=P)

            for k in range(nw):
                for t in range(nt[k]):
                    rows0 = int(offs[k]) + t * P
                    slot0 = t * P
                    is_light = light[k][t]

                    # load + cast ins lhsT K-tiles: [128, 8, 128]
                    xh = xinp.tile([P, 8, P], f16, tag="xh")
                    if "nox" in abl:
                        nc.vector.memset(xh[:], 0.0)
                    else:
                        xf = xinp.tile([P, 8, P], f32, tag="xf")
                        nc.sync.dma_start(xf[:], insT_r[:, :, rows0:rows0 + P])
                        nc.vector.tensor_copy(xh[:], xf[:])

                    mk = xinp.tile([P, 1], f32, tag="mk")
                    nc.sync.dma_start(mk[:], mask_d[rows0:rows0 + P, :])

                    def lhsT(kk):
                        if kk < 8:
                            return xh[:, kk, :]
                        c = (kk - 8) * S + slot0
                        return ht[:, c:c + P]

                    hfull = houtp.tile([P, H], f32, tag="hfull")

                    for jc in range(2):
                        nkt = 8 if is_light else 16
                        pr = psum.tile([P, 512], f32, tag="pr")
                        for kk in range(nkt):
                            nc.tensor.matmul(pr[:], lhsT(kk), wcol(kk, 0, jc),
                                             start=(kk == 0), stop=(kk == nkt - 1))
                        pz = psum.tile([P, 512], f32, tag="pz")
                        for kk in range(nkt):
                            nc.tensor.matmul(pz[:], lhsT(kk), wcol(kk, 1, jc),
                                             start=(kk == 0), stop=(kk == nkt - 1))
                        pin = psum.tile([P, 512], f32, tag="pin")
                        for kk in range(8):
                            nc.tensor.matmul(pin[:], xh[:, kk, :], wcol(kk, 2, jc),
                                             start=(kk == 0), stop=(kk == 7))
                        if not is_light:
                            phn = psum.tile([P, 512], f32, tag="phn")
                            for kk in range(8):
                                nc.tensor.matmul(phn[:], lhsT(kk + 8),
                                                 wcol(kk + 8, 2, jc),
                                                 start=(kk == 0), stop=(kk == 7))

                        if use_bi:
                            nc.vector.tensor_add(pr[:], pr[:],
                                                 bib[:, 0 * H + jc * 512:0 * H + jc * 512 + 512])
                            nc.vector.tensor_add(pz[:], pz[:],
                                                 bib[:, 1 * H + jc * 512:1 * H + jc * 512 + 512])
                            nc.vector.tensor_add(pin[:], pin[:],
                                                 bib[:, 2 * H + jc * 512:2 * H + jc * 512 + 512])

                        if "nogates" in abl:
                            continue
                        r_t = gatesp.tile([P, 512], f32, tag="r")
                        nc.scalar.activation(r_t[:], pr[:], AF.Sigmoid)
                        z_t = gatesp.tile([P, 512], f32, tag="z")
                        nc.scalar.activation(z_t[:], pz[:], AF.Sigmoid)

                        n_t = gatesp.tile([P, 512], f32, tag="n")
                        if is_light:
                            # h == 0: n = tanh(i_n + r * b_hn)
                            if use_bhn:
                                t1 = gatesp.tile([P, 512], f32, tag="t1")
                                nc.vector.tensor_mul(
                                    t1[:], r_t[:], bhnb[:, jc * 512:jc * 512 + 512])
                                t2 = gatesp.tile([P, 512], f32, tag="t2")
                                nc.vector.tensor_add(t2[:], t1[:], pin[:])
                                nc.scalar.activation(n_t[:], t2[:], AF.Tanh)
                            else:
                                nc.scalar.activation(n_t[:], pin[:], AF.Tanh)
                            # h = (1-z)*n = n - z*n
                            zs = gatesp.tile([P, 512], f32, tag="zs")
                            nc.vector.tensor_mul(zs[:], z_t[:], n_t[:])
                            nc.vector.tensor_sub(
                                hfull[:, jc * 512:jc * 512 + 512], n_t[:], zs[:])
                        else:
                            if use_bhn:
                                nc.vector.tensor_add(
                                    phn[:], phn[:], bhnb[:, jc * 512:jc * 512 + 512])
                            t1 = gatesp.tile([P, 512], f32, tag="t1")
                            nc.vector.tensor_mul(t1[:], r_t[:], phn[:])
                            t2 = gatesp.tile([P, 512], f32, tag="t2")
                            nc.vector.tensor_add(t2[:], t1[:], pin[:])
                            nc.scalar.activation(n_t[:], t2[:], AF.Tanh)
                            # h = n + z*(h_prev - n)
                            hp = hr[:, t * H + jc * 512:t * H + jc * 512 + 512]
                            s_t = gatesp.tile([P, 512], f32, tag="s")
                            nc.vector.tensor_sub(s_t[:], hp, n_t[:])
                            zs = gatesp.tile([P, 512], f32, tag="zs")
                            nc.vector.tensor_mul(zs[:], z_t[:], s_t[:])
                            nc.vector.tensor_add(
                                hfull[:, jc * 512:jc * 512 + 512], n_t[:], zs[:])

                        # masked fp16 state update (row-major)
                        nc.vector.tensor_scalar_mul(
                            hr[:, t * H + jc * 512:t * H + jc * 512 + 512],
                            hfull[:, jc * 512:jc * 512 + 512], mk[:])

                    # output
                    if "noy" not in abl and "nogates" not in abl:
                        nc.scalar.dma_start(y_d[rows0:rows0 + P, :], hfull[:])

                    # transposed state update for next wave (DMA xbar
                    # transpose into a contiguous staging tile; gpsimd copies
                    # it into the persistent H_T buffer)
                    if (k + 1 < nw and t < nt[k + 1] and "notrans" not in abl
                            and "nogates" not in abl):
                        for kk in range(8):
                            hr_sl = hr[:, t * H + kk * P:t * H + (kk + 1) * P]
                            ht_sl = ht[:, kk * S + slot0:kk * S + slot0 + P]
                            tp = psum.tile([P, P], f32, tag="pin")
                            nc.tensor.transpose(tp[:], hr_sl, ident[:])
                            nc.scalar.copy(ht_sl, tp[:])

            rep_ctx.close()

    nc.compile()
    return nc


# ----------------------------------------------------------------------------
# Entry point
# ----------------------------------------------------------------------------

def kernel(ins, resets, W_i, b_i, W_h, b_hn):
    from concourse.bass_utils import run_bass_kernel_spmd

    t_host0 = time.time()
    ins = np.asarray(ins, dtype=np.float32)
    resets = np.asarray(resets).astype(bool)
    W_i = np.ascontiguousarray(np.asarray(W_i, dtype=np.float32))
    W_h = np.ascontiguousarray(np.asarray(W_h, dtype=np.float32))
    b_i = np.asarray(b_i, dtype=np.float32)
    b_hn = np.asarray(b_hn, dtype=np.float32)

    use_bi = bool(np.any(b_i != 0))
    use_bhn = bool(np.any(b_hn != 0))

    sched = _schedule(resets)
    r_total = sched["r_total"]

    t_build0 = time.time()
    nc = _build_nc(sched, use_bi, use_bhn)
    t_build = time.time() - t_build0

    ins_flat = ins.reshape(T * B, H)
    in_maps = []
    for c in range(NCORES):
        src = sched["cores"][c]["src"]
        a = np.zeros((r_total, H), dtype=np.float32)
        valid = src >= 0
        a[valid] = ins_flat[src[valid]]
        im = {
            "insT": np.ascontiguousarray(a.T),
            "Wi": W_i,
            "Wh": W_h,
            "mask": sched["cores"][c]["mask"].reshape(r_total, 1),
            "ident": np.eye(P, dtype=np.float32),
        }
        if use_bi:
            im["bib"] = np.broadcast_to(b_i, (P, 3 * H)).copy()
        if use_bhn:
            im["bhnb"] = np.broadcast_to(b_hn, (P, H)).copy()
        in_maps.append(im)

    t_run0 = time.time()
    res = run_bass_kernel_spmd(nc, in_maps, list(range(NCORES)))
    t_run = time.time() - t_run0

    ys_flat = np.zeros((T * B, H), dtype=np.float32)
    for c in range(NCORES):
        ydst = sched["cores"][c]["ydst"]
        valid = ydst >= 0
        ys_flat[ydst[valid]] = res.results[c]["Y"][valid]

    _rt_info.update(
        sched_units=sched["n_units"], r_total=r_total, nw=sched["nw"],
        nt=sched["nt"], t_build=t_build, t_run=t_run,
        t_host=time.time() - t_host0, nc=nc, in_maps=in_maps,
        exec_time_ns=res.exec_time_ns, sched=sched,
        use_bi=use_bi, use_bhn=use_bhn,
    )
    return ys_flat.reshape(T, B, H)


# ----------------------------------------------------------------------------
# Benchmarking (wall-clock of the compiled executable; dispatch overhead is
# estimated with a stub NEFF and subtracted)
# ----------------------------------------------------------------------------

def _make_callable(nc, in_maps, donate=False):
    """jit'd shard_map callable over the prebuilt bass module (all 8 cores).

    Returns (fn, dev_args): call fn(*dev_args) -> tuple of outputs.
    """
    import jax
    import numpy as np
    from jax.experimental.shard_map import shard_map
    from jax.sharding import Mesh, PartitionSpec

    from concourse import mybir
    from concourse.bass2jax import (
        _bass_exec_p,
        install_neuronx_cc_hook,
        partition_id_tensor,
    )

    install_neuronx_cc_hook()
    n_cores = len(in_maps)
    partition_name = (nc.partition_id_tensor.name
                      if nc.partition_id_tensor else None)

    in_names, out_names, out_avals, zero_outs = [], [], [], []
    for alloc in nc.m.functions[0].allocations:
        if not isinstance(alloc, mybir.MemoryLocationSet):
            continue
        name = alloc.memorylocations[0].name
        if alloc.kind == "ExternalInput":
            if name != partition_name:
                in_names.append(name)
        elif alloc.kind == "ExternalOutput":
            out_names.append(name)
            shape = tuple(alloc.tensor_shape)
            dtype = mybir.dt.np(alloc.dtype)
            out_avals.append(jax.core.ShapedArray(shape, dtype))
            zero_outs.append(np.zeros(shape, dtype))
    n_params = len(in_names)
    all_names = in_names + out_names
    if partition_name is not None:
        all_names = all_names + [partition_name]

    def _body(*args):
        operands = list(args)
        if partition_name is not None:
            operands.append(partition_id_tensor())
        outs = _bass_exec_p.bind(
            *operands,
            out_avals=tuple(out_avals),
            in_names=tuple(all_names),
            out_names=tuple(out_names),
            lowering_input_output_aliases=(),
            sim_require_finite=True,
            sim_require_nnan=True,
            nc=nc,
        )
        return tuple(outs)

    devices = jax.devices()[:n_cores]
    mesh = Mesh(np.asarray(devices), ("core",))
    specs = (PartitionSpec("core"),) * (n_params + len(out_names))
    fn = jax.jit(
        shard_map(_body, mesh=mesh, in_specs=specs,
                  out_specs=(PartitionSpec("core"),) * len(out_names),
                  check_rep=False),
        donate_argnums=(tuple(range(n_params, n_params + len(out_names)))
                        if donate else ()),
        keep_unused=True,
    )
    concat = [
        np.concatenate([np.asarray(in_maps[c][n]) for c in range(n_cores)],
                       axis=0)
        for n in in_names
    ] + [np.zeros((n_cores * z.shape[0], *z.shape[1:]), z.dtype)
         for z in zero_outs]
    dev_args = [jax.device_put(a) for a in concat]
    return fn, dev_args


def bench(n=6):
    """Returns dict with wall-clock stats for the last-built kernel."""
    import jax

    assert "nc" in _rt_info, "run kernel() first"

    def time_calls(fn, args, n):
        ts = []
        for _ in range(n):
            t0 = time.perf_counter()
            jax.block_until_ready(fn(*args))
            ts.append(time.perf_counter() - t0)
        return ts

    fn, dev_args = _make_callable(_rt_info["nc"], _rt_info["in_maps"])
    t_real = time_calls(fn, dev_args, n)

    # dispatch-overhead stub: trivial NEFF with tiny I/O
    stub = _stub_nc()
    stub_maps = [{"sx": np.zeros((P, P), np.float32)} for _ in range(NCORES)]
    fn_s, dev_s = _make_callable(stub, stub_maps)
    t_stub = time_calls(fn_s, dev_s, n)

    t_real_med = sorted(t_real)[len(t_real) // 2]
    t_stub_med = sorted(t_stub)[len(t_stub) // 2]
    est = (t_real_med - t_stub_med) * 1e9
    out = {
        "t_real_ms": [round(t * 1e3, 3) for t in t_real],
        "t_stub_ms": [round(t * 1e3, 3) for t in t_stub],
        "hw_exec_est_ns": int(est),
    }
    _rt_info["bench"] = out
    return out


def bench_hw(r_hi=129, n=12):
    """HW per-iteration time via in-NEFF repeat loop slope (robust to RPC
    jitter). Requires kernel() to have been called (schedule cached)."""
    import jax

    sched = _rt_info["sched"]
    in_maps = _rt_info["in_maps"]

    def timed(nc):
        fn, dev_args = _make_callable(nc, in_maps)
        ts = []
        for _ in range(n):
            t0 = time.perf_counter()
            jax.block_until_ready(fn(*dev_args))
            ts.append(time.perf_counter() - t0)
        ts.sort()
        return ts

    t_build0 = time.time()
    nc1 = _build_nc(sched, _rt_info["use_bi"], _rt_info["use_bhn"], repeat=1)
    nc_hi = _build_nc(sched, _rt_info["use_bi"], _rt_info["use_bhn"],
                      repeat=r_hi)
    print(f"bench_hw builds: {time.time() - t_build0:.1f}s")
    ts1 = timed(nc1)
    ts_hi = timed(nc_hi)
    med1 = ts1[len(ts1) // 2]
    med_hi = ts_hi[len(ts_hi) // 2]
    per_iter_ns = (med_hi - med1) / (r_hi - 1) * 1e9
    out = {
        "t1_ms": [round(t * 1e3, 2) for t in ts1],
        "thi_ms": [round(t * 1e3, 2) for t in ts_hi],
        "r_hi": r_hi,
        "hw_exec_ns": int(per_iter_ns),
    }
    _rt_info["bench_hw"] = out
    return out


def _stub_nc():
    import concourse.tile as tile
    from concourse import bacc, mybir

    nc = bacc.Bacc("TRN2", target_bir_lowering=False, debug=False,
                   num_devices=NCORES)
    x_d = nc.dram_tensor("sx", [P, P], mybir.dt.float32,
                         kind="ExternalInput").ap()
    y_d = nc.dram_tensor("sy", [P, P], mybir.dt.float32,
                         kind="ExternalOutput").ap()
    with tile.TileContext(nc) as tc:
        with tc.tile_pool(name="p", bufs=1) as pool:
            t = pool.tile([P, P], mybir.dt.float32)
            nc.sync.dma_start(t[:], x_d[:, :])
            nc.sync.dma_start(y_d[:, :], t[:])

    nc.compile()
    return nc
